# revision 6
# baseline (speedup 1.0000x reference)
"""Trainium2 Bass kernel: 8-connectivity connected-component labeling of a
4096x4096 binary image (prob > 0.5); labels = min linear index in component
+ 1, background 0 (int32).

Strategy (single device launch + tiny host merge):
  - Shard rows: 8 cores x 512-row strips. Each core solves CCL exactly
    within 64x128 tiles of its strip: iterate [row segmented scans fwd/bwd,
    3x3 conduit window-min (clamped at tile borders), col segmented scans
    fwd/bwd] in a For_i hardware loop to a guaranteed tile-local fixpoint
    (iteration count measured for this regime + margin), then one unrolled
    verification iteration computes an on-device change flag against a
    DRAM snapshot.
  - Host: union-find over tile-seam label equivalences (pure numpy) and a
    LUT remap to global component minima. Exact two-level CCL.
  - If the device flag reports non-convergence (never for this regime), the
    host finishes the remaining iterations in numpy before merging, so the
    result is exact for any input.
"""
import sys
sys.path.insert(0, '/opt/trn_rl_repo')
sys.path.insert(0, '/root/.axon_site')
sys.path.insert(0, '/root/.axon_site/_ro/trn_rl_repo')
import numpy as np

import concourse.bass as bass
import concourse.bacc as bacc
import concourse.mybir as mybir
import concourse.tile as tile
from concourse import masks as cmasks
from concourse.bass_utils import run_bass_kernel_spmd

F32 = mybir.dt.float32
I32 = mybir.dt.int32
U8 = mybir.dt.uint8
AL = mybir.AluOpType
AX = mybir.AxisListType

H = W = 4096
NCORES = 8
SR = H // NCORES            # 512 rows per core
NB = SR // 128              # 4 R-tiles (128 rows x 4096)
NT = W // 128               # 32 T-tiles (128 cols x 512)
TY = 64                     # tile rows (gate in T-form free dim)
TX = 128                    # tile cols (= transpose block, clamped hmin3)
NIT = 112                   # For_i iterations (worst measured 84 + margin)
BIG = float(2 ** 25)
BIGI = np.int64(2 ** 25)


def kernel_body(tc, outs, ins, prev):
    nc = tc.nc
    from contextlib import ExitStack
    ctx = ExitStack()
    with ctx:
        pool = ctx.enter_context(tc.tile_pool(name="main", bufs=1))
        tmp = ctx.enter_context(tc.tile_pool(name="tmp", bufs=2))
        psum = ctx.enter_context(tc.tile_pool(name="ps", bufs=4, space="PSUM"))

        ident = pool.tile([128, 128], F32)
        cmasks.make_identity(nc, ident[:])

        LR = [pool.tile([128, W], F32, name=f"LR{b}") for b in range(NB)]
        FG = [pool.tile([128, W], U8, name=f"FG{b}") for b in range(NB)]
        GRb = [pool.tile([128, W], U8, name=f"GRb{b}") for b in range(NB)]
        GCb = [pool.tile([128, SR], U8, name=f"GCb{t}") for t in range(NT)]
        fgT = [pool.tile([128, SR], U8, name=f"fgT{t}") for t in range(NT)]
        GVT = pool.tile([128, SR], F32, name="GVT")
        t1 = pool.tile([128, W], F32, name="t1")
        io = pool.tile([128, W], I32, name="io")
        offt = pool.tile([128, 1], F32, name="offt")
        flag = pool.tile([128, 1], F32, name="flag")
        fr = pool.tile([128, 1], F32, name="fr")

        fg_r = ins["fg"].rearrange("(a p) w -> a p w", p=128)
        lab_r = outs["lab"].rearrange("(a p) w -> a p w", p=128)
        prev_r = prev.rearrange("(a p) w -> a p w", p=128)

        # ---- init ----
        nc.sync.dma_start(offt[:], ins["off"])
        nc.vector.tensor_scalar(flag[:], flag[:], 0.0, 0.0, op0=AL.mult,
                                op1=AL.mult)
        # GVT: BIG where y % TY == 0 else 0 (vmin3 tile-row boundary gate)
        nc.vector.tensor_scalar(GVT[:], GVT[:], 0.0, 0.0, op0=AL.mult,
                                op1=AL.mult)
        gv3 = GVT[:].rearrange("p (a b) -> p a b", b=TY)
        nc.gpsimd.affine_select(gv3, gv3, pattern=[[0, SR // TY], [1, TY]],
                                compare_op=AL.is_gt, fill=BIG,
                                base=0, channel_multiplier=0)
        for b in range(NB):
            nc.sync.dma_start(FG[b][:], fg_r[b])
            # labels = local linear index + 1 + core offset, bg -> BIG
            nc.gpsimd.iota(io[:], pattern=[[1, W]], base=b * 128 * W,
                           channel_multiplier=W)
            nc.vector.tensor_scalar(LR[b][:], io[:], 1.0, 0.0, op0=AL.add,
                                    op1=AL.add)
            off_b = offt[:].broadcast_to([128, W])
            nc.vector.tensor_tensor(LR[b][:], LR[b][:], off_b, op=AL.add)
            nc.vector.tensor_copy(t1[:], FG[b][:])
            nc.vector.tensor_scalar(t1[:], t1[:], -BIG, BIG, op0=AL.mult,
                                    op1=AL.add)
            nc.vector.tensor_tensor(LR[b][:], LR[b][:], t1[:], op=AL.max)
            # row edge gate bool: fg[x-1] & fg[x] & (x % TX != 0)
            nc.vector.tensor_tensor(GRb[b][:, 1:], FG[b][:, 1:],
                                    FG[b][:, :-1], op=AL.mult)
            nc.vector.tensor_scalar(GRb[b][:, :1], FG[b][:, :1], 0.0, 0.0,
                                    op0=AL.mult, op1=AL.mult)
            nc.vector.tensor_copy(t1[:], GRb[b][:])
            g3 = t1[:].rearrange("p (a b) -> p a b", b=TX)
            nc.gpsimd.affine_select(g3, g3, pattern=[[0, W // TX], [1, TX]],
                                    compare_op=AL.is_gt, fill=0.0,
                                    base=0, channel_multiplier=0)
            nc.vector.tensor_copy(GRb[b][:], t1[:])
        # T-form fg + col gates
        for t in range(NT):
            pin = psum.tile([128, SR], F32, tag="pin")
            for b in range(NB):
                hbf = tmp.tile([128, 128], F32, tag="hbf")
                nc.vector.tensor_copy(hbf[:], FG[b][:, t * 128:(t + 1) * 128])
                nc.tensor.transpose(pin[:, b * 128:(b + 1) * 128],
                                    hbf[:], ident[:])
            ft = tmp.tile([128, SR], F32, tag="ft")
            nc.scalar.copy(ft[:], pin[:])
            nc.vector.tensor_copy(fgT[t][:], ft[:])
            # col edge gate bool: fg[y-1] & fg[y] & (y % TY != 0)
            gt = tmp.tile([128, SR], F32, tag="gt")
            nc.vector.tensor_tensor(gt[:, 1:], ft[:, 1:], ft[:, :-1],
                                    op=AL.mult)
            nc.vector.tensor_scalar(gt[:, :1], ft[:, :1], 0.0, 0.0,
                                    op0=AL.mult, op1=AL.mult)
            gt3 = gt[:].rearrange("p (a b) -> p a b", b=TY)
            nc.gpsimd.affine_select(gt3, gt3, pattern=[[0, SR // TY], [1, TY]],
                                    compare_op=AL.is_gt, fill=0.0,
                                    base=0, channel_multiplier=0)
            nc.vector.tensor_copy(GCb[t][:], gt[:])

        # ---- one solve iteration (in-place on LR) ----
        def iteration():
            # row scans (per R-tile), gates derived from GRb
            for b in range(NB):
                nc.vector.tensor_copy(t1[:], GRb[b][:])
                nc.vector.tensor_scalar(t1[:], t1[:], -BIG, BIG,
                                        op0=AL.mult, op1=AL.add)
                nc.vector.tensor_tensor_scan(LR[b][:], t1[:], LR[b][:],
                                             BIG, op0=AL.max, op1=AL.min)
                nc.vector.tensor_tensor_scan(
                    LR[b][:, W - 2::-1], t1[:, W - 1:0:-1],
                    LR[b][:, W - 2::-1], LR[b][:, W - 1:W],
                    op0=AL.max, op1=AL.min)
            # T pass
            for t in range(NT):
                pin = psum.tile([128, SR], F32, tag="pin")
                for b in range(NB):
                    blk = LR[b][:, t * 128:(t + 1) * 128]
                    hb = tmp.tile([128, 128], F32, tag="hb")
                    nc.scalar.copy(hb[:], blk)
                    nc.vector.tensor_tensor(hb[:, 1:], hb[:, 1:], blk[:, :-1],
                                            op=AL.min)
                    nc.vector.tensor_tensor(hb[:, :-1], hb[:, :-1],
                                            blk[:, 1:], op=AL.min)
                    nc.tensor.transpose(pin[:, b * 128:(b + 1) * 128],
                                        hb[:], ident[:])
                tb = tmp.tile([128, SR], F32, tag="tb")
                nc.scalar.copy(tb[:], pin[:])
                # vmin3 conduit, gated at tile-row boundary (pristine = pin)
                va = tmp.tile([128, SR], F32, tag="va")
                nc.vector.tensor_tensor(va[:, 1:], pin[:, :-1], GVT[:, 1:],
                                        op=AL.max)
                nc.vector.tensor_tensor(tb[:, 1:], tb[:, 1:], va[:, 1:],
                                        op=AL.min)
                nc.vector.tensor_tensor(va[:, :-1], pin[:, 1:], GVT[:, 1:],
                                        op=AL.max)
                nc.vector.tensor_tensor(tb[:, :-1], tb[:, :-1], va[:, :-1],
                                        op=AL.min)
                # restore bg to BIG
                nc.vector.tensor_copy(va[:], fgT[t][:])
                nc.vector.tensor_scalar(va[:], va[:], -BIG, BIG,
                                        op0=AL.mult, op1=AL.add)
                nc.vector.tensor_tensor(tb[:], tb[:], va[:], op=AL.max)
                # col scans
                gc = tmp.tile([128, SR], F32, tag="gc")
                nc.vector.tensor_copy(gc[:], GCb[t][:])
                nc.vector.tensor_scalar(gc[:], gc[:], -BIG, BIG,
                                        op0=AL.mult, op1=AL.add)
                nc.vector.tensor_tensor_scan(tb[:], gc[:], tb[:], BIG,
                                             op0=AL.max, op1=AL.min)
                nc.vector.tensor_tensor_scan(
                    tb[:, SR - 2::-1], gc[:, SR - 1:0:-1],
                    tb[:, SR - 2::-1], tb[:, SR - 1:SR],
                    op0=AL.max, op1=AL.min)
                # transpose back
                pout = psum.tile([128, SR], F32, tag="pout")
                for b in range(NB):
                    nc.tensor.transpose(pout[:, b * 128:(b + 1) * 128],
                                        tb[:, b * 128:(b + 1) * 128],
                                        ident[:])
                for b in range(NB):
                    nc.scalar.copy(LR[b][:, t * 128:(t + 1) * 128],
                                   pout[:, b * 128:(b + 1) * 128])

        with tc.For_i(0, NIT, 1):
            iteration()
        # snapshot, one more iteration, compare -> convergence flag
        for b in range(NB):
            nc.sync.dma_start(prev_r[b], LR[b][:])
        iteration()
        for b in range(NB):
            nc.sync.dma_start(t1[:], prev_r[b])
            nc.vector.tensor_tensor(t1[:], t1[:], LR[b][:], op=AL.not_equal)
            nc.vector.tensor_reduce(fr[:], t1[:], axis=AX.X, op=AL.max)
            nc.vector.tensor_tensor(flag[:], flag[:], fr[:], op=AL.max)

        # ---- epilogue: labels to i32 (bg -> 0), flag out ----
        for b in range(NB):
            nc.vector.tensor_scalar(t1[:], LR[b][:], BIG, 0.0, op0=AL.is_lt,
                                    op1=AL.add)
            nc.vector.tensor_tensor(io[:], LR[b][:], t1[:], op=AL.mult)
            nc.sync.dma_start(lab_r[b], io[:])
        nc.sync.dma_start(outs["flag"], flag[:])


def build_program():
    nc = bacc.Bacc("TRN2", target_bir_lowering=False, debug=False,
                   num_devices=NCORES)
    ins = {
        "fg": nc.dram_tensor("fg", [SR, W], U8, kind="ExternalInput").ap(),
        "off": nc.dram_tensor("off", [128, 1], F32,
                              kind="ExternalInput").ap(),
    }
    outs = {
        "lab": nc.dram_tensor("lab", [SR, W], I32,
                              kind="ExternalOutput").ap(),
        "flag": nc.dram_tensor("flag", [128, 1], F32,
                               kind="ExternalOutput").ap(),
    }
    prev = nc.dram_tensor("prev", [SR, W], F32, kind="Internal").ap()
    with tile.TileContext(nc) as tc:
        kernel_body(tc, outs, ins, prev)
    nc.compile()
    return nc


# ---------------------------------------------------------------------------
# host side
# ---------------------------------------------------------------------------

def _host_finish(lab, fg):
    """Finish tile-local solves in numpy if the device flag fired (monotone
    continuation of the same operator; exact for any input)."""
    B = ~fg
    L = np.where(fg, lab.astype(np.int64), BIGI)
    gR = np.full((H, W), BIGI)
    m = fg[:, 1:] & fg[:, :-1]
    gR[:, 1:] = np.where(m, 0, BIGI)
    gR[:, 0::TX] = BIGI
    gC = np.full((H, W), BIGI)
    m = fg[1:, :] & fg[:-1, :]
    gC[1:, :] = np.where(m, 0, BIGI)
    for r in range(0, H, SR):   # strip borders are tile borders too
        gC[r, :] = BIGI
    gC[0::TY, :] = BIGI

    def scan(Lm, G, axis, reverse):
        if axis == 0:
            Lm = Lm.T; G = G.T
        h, w = Lm.shape
        if reverse:
            Lf = Lm[:, ::-1]
            Gf = np.empty_like(G)
            Gf[:, :w - 1] = G[:, 1:][:, ::-1]
            Gf[:, w - 1] = BIGI
        else:
            Lf, Gf = Lm, G
        seg = np.cumsum(Gf >= BIGI, axis=1)
        sp = (w + 2) - seg
        K = np.int64(1 << 32)
        C = np.minimum.accumulate(Lf + sp * K, axis=1)
        res = np.minimum(C - sp * K, Lf)
        if reverse:
            res = res[:, ::-1]
        return res.T if axis == 0 else res

    for _ in range(2000):
        prev = L
        L = scan(L, gR, 1, False)
        L = scan(L, gR, 1, True)
        Lh = L.reshape(H, W // TX, TX)
        a = np.full_like(Lh, BIGI); a[:, :, 1:] = Lh[:, :, :-1]
        b2 = np.full_like(Lh, BIGI); b2[:, :, :-1] = Lh[:, :, 1:]
        Hm = np.minimum(Lh, np.minimum(a, b2)).reshape(H, W)
        Hv = Hm.reshape(H // TY, TY, W)
        a = np.full_like(Hv, BIGI); a[:, 1:, :] = Hv[:, :-1, :]
        b2 = np.full_like(Hv, BIGI); b2[:, :-1, :] = Hv[:, 1:, :]
        L = np.where(B, BIGI, np.minimum(Hv, np.minimum(a, b2)).reshape(H, W))
        L = scan(L, gC, 0, False)
        L = scan(L, gC, 0, True)
        if np.array_equal(L, prev):
            break
    return np.where(B, 0, L).astype(np.int32)


def _merge(lab, fg):
    """Union tile-seam equivalences (pure numpy), remap to component minima."""
    pairs = []

    def add(la, lb, fa, fb):
        m = fa & fb
        if m.any():
            pairs.append(np.stack([la[m], lb[m]], 1))
        m = fa[1:] & fb[:-1]
        if m.any():
            pairs.append(np.stack([la[1:][m], lb[:-1][m]], 1))
        m = fa[:-1] & fb[1:]
        if m.any():
            pairs.append(np.stack([la[:-1][m], lb[1:][m]], 1))

    for r in range(TY - 1, H - 1, TY):
        add(lab[r], lab[r + 1], fg[r], fg[r + 1])
    for c in range(TX - 1, W - 1, TX):
        add(lab[:, c], lab[:, c + 1], fg[:, c], fg[:, c + 1])
    if not pairs:
        return lab
    P = np.concatenate(pairs)
    uniq, inv = np.unique(P, return_inverse=True)
    inv = inv.reshape(-1, 2)
    e0, e1 = inv[:, 0], inv[:, 1]
    par = np.arange(len(uniq), dtype=np.int64)
    while True:
        a = par[e0]; b = par[e1]
        if (a == b).all():
            break
        m = np.minimum(a, b)
        np.minimum.at(par, e0, m)
        np.minimum.at(par, e1, m)
        par = par[par[par]]
    LUT = np.arange(H * W + 1, dtype=np.int32)
    LUT[uniq] = uniq[par].astype(np.int32)
    return LUT[lab]


_CACHED = {}


def kernel(prob):
    prob2 = np.squeeze(np.asarray(prob))
    fg = prob2 > 0.5

    if 'nc' not in _CACHED:
        _CACHED['nc'] = build_program()
    nc = _CACHED['nc']

    in_maps = []
    for c in range(NCORES):
        in_maps.append({
            "fg": fg[c * SR:(c + 1) * SR].astype(np.uint8),
            "off": np.full((128, 1), float(c * SR * W), np.float32),
        })
    res = run_bass_kernel_spmd(nc, in_maps, core_ids=list(range(NCORES)))
    kernel._launches = 1

    lab = np.concatenate([res.results[c]["lab"] for c in range(NCORES)], 0)
    converged = all(float(res.results[c]["flag"].max()) == 0.0
                    for c in range(NCORES))
    if not converged:
        lab = _host_finish(lab, fg)
    return _merge(lab, fg)


# revision 8
# speedup vs baseline: 1.8440x; 1.8440x over previous
"""Trainium2 Bass kernel: 8-connectivity connected-component labeling of a
4096x4096 binary image (prob > 0.5); labels = min linear index in component
+ 1, background 0 (int32).

Strategy (single device launch + tiny host merge):
  - Shard rows: 8 cores x 512-row strips. Each core solves CCL exactly
    within 64x128 tiles of its strip: iterate [row segmented scans fwd/bwd,
    3x3 conduit window-min (clamped at tile borders), col segmented scans
    fwd/bwd] in a For_i hardware loop to a guaranteed tile-local fixpoint
    (iteration count measured for this regime + margin), then one unrolled
    verification iteration computes an on-device change flag against a
    DRAM snapshot.
  - Host: union-find over tile-seam label equivalences (pure numpy) and a
    LUT remap to global component minima. Exact two-level CCL.
  - If the device flag reports non-convergence (never for this regime), the
    host finishes the remaining iterations in numpy before merging, so the
    result is exact for any input.
"""
import sys
sys.path.insert(0, '/opt/trn_rl_repo')
sys.path.insert(0, '/root/.axon_site')
sys.path.insert(0, '/root/.axon_site/_ro/trn_rl_repo')
import numpy as np

import concourse.bass as bass
import concourse.bacc as bacc
import concourse.mybir as mybir
import concourse.tile as tile
from concourse import masks as cmasks
from concourse.bass_utils import run_bass_kernel_spmd

F32 = mybir.dt.float32
I32 = mybir.dt.int32
U8 = mybir.dt.uint8
AL = mybir.AluOpType
AX = mybir.AxisListType

H = W = 4096
NCORES = 8
SR = H // NCORES            # 512 rows per core
NB = SR // 128              # 4 R-tiles (128 rows x 4096)
NT = W // 128               # 32 T-tiles (128 cols x 512)
TY = 64                     # tile rows (gate in T-form free dim)
TX = 128                    # tile cols (= transpose block, clamped hmin3)
NIT = 112                   # For_i iterations (worst measured 84 + margin)
BIG = float(2 ** 25)
BIGI = np.int64(2 ** 25)


def kernel_body(tc, outs, ins, prev):
    nc = tc.nc
    from contextlib import ExitStack
    ctx = ExitStack()
    with ctx:
        pool = ctx.enter_context(tc.tile_pool(name="main", bufs=1))
        tmp = ctx.enter_context(tc.tile_pool(name="tmp", bufs=2))
        psum = ctx.enter_context(tc.tile_pool(name="ps", bufs=4, space="PSUM"))

        ident = pool.tile([128, 128], F32)
        cmasks.make_identity(nc, ident[:])

        LR = [pool.tile([128, W], F32, name=f"LR{b}") for b in range(NB)]
        FG = [pool.tile([128, W], U8, name=f"FG{b}") for b in range(NB)]
        GRb = [pool.tile([128, W], U8, name=f"GRb{b}") for b in range(NB)]
        GCb = [pool.tile([128, SR], U8, name=f"GCb{t}") for t in range(NT)]
        fgT = [pool.tile([128, SR], U8, name=f"fgT{t}") for t in range(NT)]
        GVT = pool.tile([128, SR], F32, name="GVT")
        t1 = pool.tile([128, W], F32, name="t1")
        io = pool.tile([128, W], I32, name="io")
        offt = pool.tile([128, 1], F32, name="offt")
        flag = pool.tile([128, 1], F32, name="flag")
        fr = pool.tile([128, 1], F32, name="fr")

        fg_r = ins["fg"].rearrange("(a p) w -> a p w", p=128)
        lab_r = outs["lab"].rearrange("(a p) w -> a p w", p=128)
        prev_r = prev.rearrange("(a p) w -> a p w", p=128)

        # ---- init ----
        nc.sync.dma_start(offt[:], ins["off"])
        nc.vector.tensor_scalar(flag[:], flag[:], 0.0, 0.0, op0=AL.mult,
                                op1=AL.mult)
        # GVT: BIG where y % TY == 0 else 0 (vmin3 tile-row boundary gate)
        nc.vector.tensor_scalar(GVT[:], GVT[:], 0.0, 0.0, op0=AL.mult,
                                op1=AL.mult)
        gv3 = GVT[:].rearrange("p (a b) -> p a b", b=TY)
        nc.gpsimd.affine_select(gv3, gv3, pattern=[[0, SR // TY], [1, TY]],
                                compare_op=AL.is_gt, fill=BIG,
                                base=0, channel_multiplier=0)
        for b in range(NB):
            nc.sync.dma_start(FG[b][:], fg_r[b])
            # labels = local linear index + 1 + core offset, bg -> BIG
            nc.gpsimd.iota(io[:], pattern=[[1, W]], base=b * 128 * W,
                           channel_multiplier=W)
            nc.vector.tensor_scalar(LR[b][:], io[:], 1.0, 0.0, op0=AL.add,
                                    op1=AL.add)
            off_b = offt[:].broadcast_to([128, W])
            nc.vector.tensor_tensor(LR[b][:], LR[b][:], off_b, op=AL.add)
            nc.vector.tensor_copy(t1[:], FG[b][:])
            nc.vector.tensor_scalar(t1[:], t1[:], -BIG, BIG, op0=AL.mult,
                                    op1=AL.add)
            nc.vector.tensor_tensor(LR[b][:], LR[b][:], t1[:], op=AL.max)
            # row edge gate bool: fg[x-1] & fg[x] & (x % TX != 0)
            nc.vector.tensor_tensor(GRb[b][:, 1:], FG[b][:, 1:],
                                    FG[b][:, :-1], op=AL.mult)
            nc.vector.tensor_scalar(GRb[b][:, :1], FG[b][:, :1], 0.0, 0.0,
                                    op0=AL.mult, op1=AL.mult)
            nc.vector.tensor_copy(t1[:], GRb[b][:])
            g3 = t1[:].rearrange("p (a b) -> p a b", b=TX)
            nc.gpsimd.affine_select(g3, g3, pattern=[[0, W // TX], [1, TX]],
                                    compare_op=AL.is_gt, fill=0.0,
                                    base=0, channel_multiplier=0)
            nc.vector.tensor_copy(GRb[b][:], t1[:])
        # T-form fg + col gates
        for t in range(NT):
            pin = psum.tile([128, SR], F32, tag="pin")
            for b in range(NB):
                hbf = tmp.tile([128, 128], F32, tag="hbf")
                nc.vector.tensor_copy(hbf[:], FG[b][:, t * 128:(t + 1) * 128])
                nc.tensor.transpose(pin[:, b * 128:(b + 1) * 128],
                                    hbf[:], ident[:])
            ft = tmp.tile([128, SR], F32, tag="ft")
            nc.scalar.copy(ft[:], pin[:])
            nc.vector.tensor_copy(fgT[t][:], ft[:])
            # col edge gate bool: fg[y-1] & fg[y] & (y % TY != 0)
            gt = tmp.tile([128, SR], F32, tag="gt")
            nc.vector.tensor_tensor(gt[:, 1:], ft[:, 1:], ft[:, :-1],
                                    op=AL.mult)
            nc.vector.tensor_scalar(gt[:, :1], ft[:, :1], 0.0, 0.0,
                                    op0=AL.mult, op1=AL.mult)
            gt3 = gt[:].rearrange("p (a b) -> p a b", b=TY)
            nc.gpsimd.affine_select(gt3, gt3, pattern=[[0, SR // TY], [1, TY]],
                                    compare_op=AL.is_gt, fill=0.0,
                                    base=0, channel_multiplier=0)
            nc.vector.tensor_copy(GCb[t][:], gt[:])

        # ---- one solve iteration (in-place on LR) ----
        def iteration():
            # row scans (per R-tile), gates derived from GRb
            for b in range(NB):
                nc.vector.tensor_copy(t1[:], GRb[b][:])
                nc.vector.tensor_scalar(t1[:], t1[:], -BIG, BIG,
                                        op0=AL.mult, op1=AL.add)
                nc.vector.tensor_tensor_scan(LR[b][:], t1[:], LR[b][:],
                                             BIG, op0=AL.max, op1=AL.min)
                nc.vector.tensor_tensor_scan(
                    LR[b][:, W - 2::-1], t1[:, W - 1:0:-1],
                    LR[b][:, W - 2::-1], LR[b][:, W - 1:W],
                    op0=AL.max, op1=AL.min)
            # T pass
            for t in range(NT):
                pin = psum.tile([128, SR], F32, tag="pin")
                for b in range(NB):
                    blk = LR[b][:, t * 128:(t + 1) * 128]
                    hb = tmp.tile([128, 128], F32, tag="hb")
                    nc.scalar.copy(hb[:], blk)
                    nc.vector.tensor_tensor(hb[:, 1:], hb[:, 1:], blk[:, :-1],
                                            op=AL.min)
                    nc.vector.tensor_tensor(hb[:, :-1], hb[:, :-1],
                                            blk[:, 1:], op=AL.min)
                    nc.tensor.transpose(pin[:, b * 128:(b + 1) * 128],
                                        hb[:], ident[:])
                tb = tmp.tile([128, SR], F32, tag="tb")
                nc.scalar.copy(tb[:], pin[:])
                # vmin3 conduit, gated at tile-row boundary (pristine = pin)
                va = tmp.tile([128, SR], F32, tag="va")
                nc.vector.tensor_tensor(va[:, 1:], pin[:, :-1], GVT[:, 1:],
                                        op=AL.max)
                nc.vector.tensor_tensor(tb[:, 1:], tb[:, 1:], va[:, 1:],
                                        op=AL.min)
                nc.vector.tensor_tensor(va[:, :-1], pin[:, 1:], GVT[:, 1:],
                                        op=AL.max)
                nc.vector.tensor_tensor(tb[:, :-1], tb[:, :-1], va[:, :-1],
                                        op=AL.min)
                # restore bg to BIG
                nc.vector.tensor_copy(va[:], fgT[t][:])
                nc.vector.tensor_scalar(va[:], va[:], -BIG, BIG,
                                        op0=AL.mult, op1=AL.add)
                nc.vector.tensor_tensor(tb[:], tb[:], va[:], op=AL.max)
                # col scans
                gc = tmp.tile([128, SR], F32, tag="gc")
                nc.vector.tensor_copy(gc[:], GCb[t][:])
                nc.vector.tensor_scalar(gc[:], gc[:], -BIG, BIG,
                                        op0=AL.mult, op1=AL.add)
                nc.vector.tensor_tensor_scan(tb[:], gc[:], tb[:], BIG,
                                             op0=AL.max, op1=AL.min)
                nc.vector.tensor_tensor_scan(
                    tb[:, SR - 2::-1], gc[:, SR - 1:0:-1],
                    tb[:, SR - 2::-1], tb[:, SR - 1:SR],
                    op0=AL.max, op1=AL.min)
                # transpose back
                pout = psum.tile([128, SR], F32, tag="pout")
                for b in range(NB):
                    nc.tensor.transpose(pout[:, b * 128:(b + 1) * 128],
                                        tb[:, b * 128:(b + 1) * 128],
                                        ident[:])
                for b in range(NB):
                    nc.scalar.copy(LR[b][:, t * 128:(t + 1) * 128],
                                   pout[:, b * 128:(b + 1) * 128])

        with tc.For_i(0, NIT, 1):
            iteration()
        # snapshot, one more iteration, compare -> convergence flag
        for b in range(NB):
            nc.sync.dma_start(prev_r[b], LR[b][:])
        iteration()
        for b in range(NB):
            nc.sync.dma_start(t1[:], prev_r[b])
            nc.vector.tensor_tensor(t1[:], t1[:], LR[b][:], op=AL.not_equal)
            nc.vector.tensor_reduce(fr[:], t1[:], axis=AX.X, op=AL.max)
            nc.vector.tensor_tensor(flag[:], flag[:], fr[:], op=AL.max)

        # ---- epilogue: labels to i32 (bg -> 0), flag out ----
        for b in range(NB):
            nc.vector.tensor_scalar(t1[:], LR[b][:], BIG, 0.0, op0=AL.is_lt,
                                    op1=AL.add)
            nc.vector.tensor_tensor(io[:], LR[b][:], t1[:], op=AL.mult)
            nc.sync.dma_start(lab_r[b], io[:])
        nc.sync.dma_start(outs["flag"], flag[:])


def build_program():
    nc = bacc.Bacc("TRN2", target_bir_lowering=False, debug=False,
                   num_devices=NCORES)
    ins = {
        "fg": nc.dram_tensor("fg", [SR, W], U8, kind="ExternalInput").ap(),
        "off": nc.dram_tensor("off", [128, 1], F32,
                              kind="ExternalInput").ap(),
    }
    outs = {
        "lab": nc.dram_tensor("lab", [SR, W], I32,
                              kind="ExternalOutput").ap(),
        "flag": nc.dram_tensor("flag", [128, 1], F32,
                               kind="ExternalOutput").ap(),
    }
    prev = nc.dram_tensor("prev", [SR, W], F32, kind="Internal").ap()
    with tile.TileContext(nc) as tc:
        kernel_body(tc, outs, ins, prev)
    nc.compile()
    return nc


# ---------------------------------------------------------------------------
# host side
# ---------------------------------------------------------------------------

def _host_finish(lab, fg):
    """Finish tile-local solves in numpy if the device flag fired (monotone
    continuation of the same operator; exact for any input)."""
    B = ~fg
    L = np.where(fg, lab.astype(np.int64), BIGI)
    gR = np.full((H, W), BIGI)
    m = fg[:, 1:] & fg[:, :-1]
    gR[:, 1:] = np.where(m, 0, BIGI)
    gR[:, 0::TX] = BIGI
    gC = np.full((H, W), BIGI)
    m = fg[1:, :] & fg[:-1, :]
    gC[1:, :] = np.where(m, 0, BIGI)
    for r in range(0, H, SR):   # strip borders are tile borders too
        gC[r, :] = BIGI
    gC[0::TY, :] = BIGI

    def scan(Lm, G, axis, reverse):
        if axis == 0:
            Lm = Lm.T; G = G.T
        h, w = Lm.shape
        if reverse:
            Lf = Lm[:, ::-1]
            Gf = np.empty_like(G)
            Gf[:, :w - 1] = G[:, 1:][:, ::-1]
            Gf[:, w - 1] = BIGI
        else:
            Lf, Gf = Lm, G
        seg = np.cumsum(Gf >= BIGI, axis=1)
        sp = (w + 2) - seg
        K = np.int64(1 << 32)
        C = np.minimum.accumulate(Lf + sp * K, axis=1)
        res = np.minimum(C - sp * K, Lf)
        if reverse:
            res = res[:, ::-1]
        return res.T if axis == 0 else res

    for _ in range(2000):
        prev = L
        L = scan(L, gR, 1, False)
        L = scan(L, gR, 1, True)
        Lh = L.reshape(H, W // TX, TX)
        a = np.full_like(Lh, BIGI); a[:, :, 1:] = Lh[:, :, :-1]
        b2 = np.full_like(Lh, BIGI); b2[:, :, :-1] = Lh[:, :, 1:]
        Hm = np.minimum(Lh, np.minimum(a, b2)).reshape(H, W)
        Hv = Hm.reshape(H // TY, TY, W)
        a = np.full_like(Hv, BIGI); a[:, 1:, :] = Hv[:, :-1, :]
        b2 = np.full_like(Hv, BIGI); b2[:, :-1, :] = Hv[:, 1:, :]
        L = np.where(B, BIGI, np.minimum(Hv, np.minimum(a, b2)).reshape(H, W))
        L = scan(L, gC, 0, False)
        L = scan(L, gC, 0, True)
        if np.array_equal(L, prev):
            break
    return np.where(B, 0, L).astype(np.int32)


def _merge(lab, fg):
    """Union tile-seam equivalences (pure numpy), remap to component minima."""
    pairs = []

    def add(la, lb, fa, fb):
        m = fa & fb
        if m.any():
            pairs.append(np.stack([la[m], lb[m]], 1))
        m = fa[1:] & fb[:-1]
        if m.any():
            pairs.append(np.stack([la[1:][m], lb[:-1][m]], 1))
        m = fa[:-1] & fb[1:]
        if m.any():
            pairs.append(np.stack([la[:-1][m], lb[1:][m]], 1))

    for r in range(TY - 1, H - 1, TY):
        add(lab[r], lab[r + 1], fg[r], fg[r + 1])
    for c in range(TX - 1, W - 1, TX):
        add(lab[:, c], lab[:, c + 1], fg[:, c], fg[:, c + 1])
    if not pairs:
        return lab
    P = np.concatenate(pairs)
    uniq, inv = np.unique(P, return_inverse=True)
    inv = inv.reshape(-1, 2)
    e0, e1 = inv[:, 0], inv[:, 1]
    par = np.arange(len(uniq), dtype=np.int64)
    while True:
        a = par[e0]; b = par[e1]
        if (a == b).all():
            break
        m = np.minimum(a, b)
        np.minimum.at(par, e0, m)
        np.minimum.at(par, e1, m)
        par = par[par[par]]
    LUT = np.arange(H * W + 1, dtype=np.int32)
    LUT[uniq] = uniq[par].astype(np.int32)
    return LUT[lab]


_CACHED = {}


def _build_runner(nc):
    """Compiled executor for the Bass program (modeled on
    bass2jax.run_bass_via_pjrt): same NEFF custom call via PJRT/shard_map,
    but the jitted callable is cached across launches and the output buffers
    are zero-filled on device instead of uploaded from host."""
    import jax
    from concourse import bass2jax
    from jax.experimental.shard_map import shard_map
    from jax.sharding import Mesh, PartitionSpec

    bass2jax.install_neuronx_cc_hook()
    partition_name = (nc.partition_id_tensor.name
                      if nc.partition_id_tensor else None)
    in_names, out_names, out_avals = [], [], []
    for alloc in nc.m.functions[0].allocations:
        if not isinstance(alloc, mybir.MemoryLocationSet):
            continue
        name = alloc.memorylocations[0].name
        if alloc.kind == "ExternalInput":
            if name != partition_name:
                in_names.append(name)
        elif alloc.kind == "ExternalOutput":
            out_names.append(name)
            out_avals.append(jax.core.ShapedArray(
                tuple(alloc.tensor_shape), mybir.dt.np(alloc.dtype)))
    n_params = len(in_names)
    bind_names = tuple(in_names + out_names +
                       ([partition_name] if partition_name else []))

    def _body(*args):
        operands = list(args)
        if partition_name is not None:
            operands.append(bass2jax.partition_id_tensor())
        outs = bass2jax._bass_exec_p.bind(
            *operands, out_avals=tuple(out_avals), in_names=bind_names,
            out_names=tuple(out_names), lowering_input_output_aliases=(),
            sim_require_finite=True, sim_require_nnan=True, nc=nc)
        return tuple(outs)

    n_outs = len(out_names)
    devices = jax.devices()[:NCORES]
    mesh = Mesh(np.asarray(devices), ("core",))
    sharded = jax.jit(
        shard_map(_body, mesh=mesh,
                  in_specs=(PartitionSpec("core"),) * (n_params + n_outs),
                  out_specs=(PartitionSpec("core"),) * n_outs,
                  check_rep=False),
        donate_argnums=tuple(range(n_params, n_params + n_outs)),
        keep_unused=True)
    from jax.sharding import NamedSharding
    shardings = tuple(NamedSharding(mesh, PartitionSpec("core"))
                      for _ in range(n_outs))
    zmaker = jax.jit(
        lambda: tuple(jax.numpy.zeros((NCORES * av.shape[0],) + av.shape[1:],
                                      av.dtype) for av in out_avals),
        out_shardings=shardings)

    def run(per_core_maps):
        concat_in = [
            np.concatenate([m[name] for m in per_core_maps], axis=0)
            for name in in_names]
        out_arrs = sharded(*concat_in, *zmaker())
        for a in out_arrs:
            a.copy_to_host_async()
        res = {}
        for i, name in enumerate(out_names):
            res[name] = np.asarray(out_arrs[i]).reshape(
                NCORES, *out_avals[i].shape)
        return res

    return run


def kernel(prob):
    prob2 = np.squeeze(np.asarray(prob))
    fg = prob2 > 0.5

    if 'run' not in _CACHED:
        _CACHED['nc'] = build_program()
        _CACHED['run'] = _build_runner(_CACHED['nc'])

    in_maps = []
    for c in range(NCORES):
        in_maps.append({
            "fg": fg[c * SR:(c + 1) * SR].astype(np.uint8),
            "off": np.full((128, 1), float(c * SR * W), np.float32),
        })
    res = _CACHED['run'](in_maps)
    kernel._launches = 1

    lab = res["lab"].reshape(H, W)
    converged = float(res["flag"].max()) == 0.0
    if not converged:
        lab = _host_finish(lab, fg)
    return _merge(lab, fg)


# revision 21
# speedup vs baseline: 3.5072x; 1.9020x over previous
"""Trainium2 Bass kernel: 8-connectivity connected-component labeling of a
4096x4096 binary image (prob > 0.5); labels = min linear index in component
+ 1, background 0 (int32).

Strategy (single device launch + tiny host merge):
  - Shard rows: 8 cores x 512-row strips. Each core solves CCL exactly
    within 64x128 tiles of its strip: iterate [row segmented scans fwd/bwd,
    3x3 conduit window-min (clamped at tile borders), col segmented scans
    fwd/bwd] in a For_i hardware loop to a guaranteed tile-local fixpoint
    (iteration count measured for this regime + margin), then one unrolled
    verification iteration computes an on-device change flag against a
    DRAM snapshot.
  - Host: union-find over tile-seam label equivalences (pure numpy) and a
    LUT remap to global component minima. Exact two-level CCL.
  - If the device flag reports non-convergence (never for this regime), the
    host finishes the remaining iterations in numpy before merging, so the
    result is exact for any input.
"""
import sys
sys.path.insert(0, '/opt/trn_rl_repo')
sys.path.insert(0, '/root/.axon_site')
sys.path.insert(0, '/root/.axon_site/_ro/trn_rl_repo')
import numpy as np

import concourse.bass as bass
import concourse.bacc as bacc
import concourse.mybir as mybir
import concourse.tile as tile
from concourse import masks as cmasks
from concourse.bass_utils import run_bass_kernel_spmd

F32 = mybir.dt.float32
I32 = mybir.dt.int32
U8 = mybir.dt.uint8
U16 = mybir.dt.uint16
AL = mybir.AluOpType
AX = mybir.AxisListType

H = W = 4096
NCORES = 8
SR = H // NCORES            # 512 rows per core
NB = SR // 128              # 4 R-tiles (128 rows x 4096)
NT = W // 128               # 32 T-tiles (128 cols x 512)
TY = 64                     # tile rows (gate in T-form free dim)
TX = 128                    # tile cols (= transpose block, clamped hmin3)
NIT = 112                   # For_i iterations (worst measured 84 + margin)
BIG = float(2 ** 25)
BIGI = np.int64(2 ** 25)


def kernel_body(tc, outs, ins, prev):
    nc = tc.nc
    from contextlib import ExitStack
    ctx = ExitStack()
    with ctx:
        pool = ctx.enter_context(tc.tile_pool(name="main", bufs=1))
        tmp = ctx.enter_context(tc.tile_pool(name="tmp", bufs=2))
        psum = ctx.enter_context(tc.tile_pool(name="ps", bufs=4, space="PSUM"))

        ident = pool.tile([128, 128], F32)
        cmasks.make_identity(nc, ident[:])

        LR = [pool.tile([128, W], F32, name=f"LR{b}") for b in range(NB)]
        PFG = [pool.tile([128, W // 8], U8, name=f"PFG{b}") for b in range(NB)]
        GRb = [pool.tile([128, W], U8, name=f"GRb{b}") for b in range(NB)]
        GCb = [pool.tile([128, SR], U8, name=f"GCb{t}") for t in range(NT)]
        fgT = [pool.tile([128, SR], U8, name=f"fgT{t}") for t in range(NT)]
        GVT = pool.tile([128, SR], F32, name="GVT")
        t1 = pool.tile([128, W], F32, name="t1")
        io = pool.tile([128, W], I32, name="io")
        offt = pool.tile([128, 1], F32, name="offt")
        flag = pool.tile([128, 1], F32, name="flag")
        fr = pool.tile([128, 1], F32, name="fr")

        fg_r = ins["fgp"].rearrange("(a p) w -> a p w", p=128)
        lab_r = outs["lab"].rearrange("(a p) w -> a p w", p=128)
        prev_r = prev.rearrange("(a p) w -> a p w", p=128)

        def unpack(dst_ap, src_ap):
            """unpack bits (little order): dst[:, 8j+k] = (src[:, j]>>k)&1"""
            for k in range(8):
                nc.vector.tensor_scalar(dst_ap[:, k::8], src_ap, float(k),
                                        1.0, op0=AL.logical_shift_right,
                                        op1=AL.bitwise_and)

        # ---- init ----
        nc.sync.dma_start(offt[:], ins["off"])
        nc.vector.tensor_scalar(flag[:], flag[:], 0.0, 0.0, op0=AL.mult,
                                op1=AL.mult)
        # GVT: BIG where y % TY == 0 else 0 (vmin3 tile-row boundary gate)
        nc.vector.tensor_scalar(GVT[:], GVT[:], 0.0, 0.0, op0=AL.mult,
                                op1=AL.mult)
        gv3 = GVT[:].rearrange("p (a b) -> p a b", b=TY)
        nc.gpsimd.affine_select(gv3, gv3, pattern=[[0, SR // TY], [1, TY]],
                                compare_op=AL.is_gt, fill=BIG,
                                base=0, channel_multiplier=0)
        for b in range(NB):
            nc.sync.dma_start(PFG[b][:], fg_r[b])
            ufg = pool.tile([128, W], U8, name="ufg") if b == 0 else ufg
            unpack(ufg[:], PFG[b][:])
            # labels = local linear index + 1 + core offset, bg -> BIG
            nc.gpsimd.iota(io[:], pattern=[[1, W]], base=b * 128 * W,
                           channel_multiplier=W)
            nc.vector.tensor_scalar(LR[b][:], io[:], 1.0, 0.0, op0=AL.add,
                                    op1=AL.add)
            off_b = offt[:].broadcast_to([128, W])
            nc.vector.tensor_tensor(LR[b][:], LR[b][:], off_b, op=AL.add)
            nc.vector.tensor_copy(t1[:], ufg[:])
            nc.vector.tensor_scalar(t1[:], t1[:], -BIG, BIG, op0=AL.mult,
                                    op1=AL.add)
            nc.vector.tensor_tensor(LR[b][:], LR[b][:], t1[:], op=AL.max)
            # row edge gate bool: fg[x-1] & fg[x] & (x % TX != 0)
            nc.vector.tensor_tensor(GRb[b][:, 1:], ufg[:, 1:],
                                    ufg[:, :-1], op=AL.mult)
            nc.vector.tensor_scalar(GRb[b][:, :1], ufg[:, :1], 0.0, 0.0,
                                    op0=AL.mult, op1=AL.mult)
            nc.vector.tensor_copy(t1[:], GRb[b][:])
            g3 = t1[:].rearrange("p (a b) -> p a b", b=TX)
            nc.gpsimd.affine_select(g3, g3, pattern=[[0, W // TX], [1, TX]],
                                    compare_op=AL.is_gt, fill=0.0,
                                    base=0, channel_multiplier=0)
            nc.vector.tensor_copy(GRb[b][:], t1[:])
        # T-form fg + col gates
        for t in range(NT):
            pin = psum.tile([128, SR], F32, tag="pin")
            for b in range(NB):
                ub = tmp.tile([128, 128], U8, tag="ub")
                unpack(ub[:], PFG[b][:, t * 16:(t + 1) * 16])
                hbf = tmp.tile([128, 128], F32, tag="hbf")
                nc.vector.tensor_copy(hbf[:], ub[:])
                nc.tensor.transpose(pin[:, b * 128:(b + 1) * 128],
                                    hbf[:], ident[:])
            ft = tmp.tile([128, SR], F32, tag="ft")
            nc.scalar.copy(ft[:], pin[:])
            nc.vector.tensor_copy(fgT[t][:], ft[:])
            # col edge gate bool: fg[y-1] & fg[y] & (y % TY != 0)
            gt = tmp.tile([128, SR], F32, tag="gt")
            nc.vector.tensor_tensor(gt[:, 1:], ft[:, 1:], ft[:, :-1],
                                    op=AL.mult)
            nc.vector.tensor_scalar(gt[:, :1], ft[:, :1], 0.0, 0.0,
                                    op0=AL.mult, op1=AL.mult)
            gt3 = gt[:].rearrange("p (a b) -> p a b", b=TY)
            nc.gpsimd.affine_select(gt3, gt3, pattern=[[0, SR // TY], [1, TY]],
                                    compare_op=AL.is_gt, fill=0.0,
                                    base=0, channel_multiplier=0)
            nc.vector.tensor_copy(GCb[t][:], gt[:])

        # ---- one solve iteration (in-place on LR) ----
        def iteration():
            # row scans (per R-tile), gates derived from GRb
            for b in range(NB):
                nc.vector.tensor_copy(t1[:], GRb[b][:])
                nc.vector.tensor_scalar(t1[:], t1[:], -BIG, BIG,
                                        op0=AL.mult, op1=AL.add)
                nc.vector.tensor_tensor_scan(LR[b][:], t1[:], LR[b][:],
                                             BIG, op0=AL.max, op1=AL.min)
                nc.vector.tensor_tensor_scan(
                    LR[b][:, W - 2::-1], t1[:, W - 1:0:-1],
                    LR[b][:, W - 2::-1], LR[b][:, W - 1:W],
                    op0=AL.max, op1=AL.min)
            # T pass
            for t in range(NT):
                pin = psum.tile([128, SR], F32, tag="pin")
                for b in range(NB):
                    blk = LR[b][:, t * 128:(t + 1) * 128]
                    hb = tmp.tile([128, 128], F32, tag="hb")
                    nc.scalar.copy(hb[:], blk)
                    nc.vector.tensor_tensor(hb[:, 1:], hb[:, 1:], blk[:, :-1],
                                            op=AL.min)
                    nc.vector.tensor_tensor(hb[:, :-1], hb[:, :-1],
                                            blk[:, 1:], op=AL.min)
                    nc.tensor.transpose(pin[:, b * 128:(b + 1) * 128],
                                        hb[:], ident[:])
                tb = tmp.tile([128, SR], F32, tag="tb")
                nc.scalar.copy(tb[:], pin[:])
                # vmin3 conduit, gated at tile-row boundary (pristine = pin)
                va = tmp.tile([128, SR], F32, tag="va")
                nc.vector.tensor_tensor(va[:, 1:], pin[:, :-1], GVT[:, 1:],
                                        op=AL.max)
                nc.vector.tensor_tensor(tb[:, 1:], tb[:, 1:], va[:, 1:],
                                        op=AL.min)
                nc.vector.tensor_tensor(va[:, :-1], pin[:, 1:], GVT[:, 1:],
                                        op=AL.max)
                nc.vector.tensor_tensor(tb[:, :-1], tb[:, :-1], va[:, :-1],
                                        op=AL.min)
                # restore bg to BIG
                nc.vector.tensor_copy(va[:], fgT[t][:])
                nc.vector.tensor_scalar(va[:], va[:], -BIG, BIG,
                                        op0=AL.mult, op1=AL.add)
                nc.vector.tensor_tensor(tb[:], tb[:], va[:], op=AL.max)
                # col scans
                gc = tmp.tile([128, SR], F32, tag="gc")
                nc.vector.tensor_copy(gc[:], GCb[t][:])
                nc.vector.tensor_scalar(gc[:], gc[:], -BIG, BIG,
                                        op0=AL.mult, op1=AL.add)
                nc.vector.tensor_tensor_scan(tb[:], gc[:], tb[:], BIG,
                                             op0=AL.max, op1=AL.min)
                nc.vector.tensor_tensor_scan(
                    tb[:, SR - 2::-1], gc[:, SR - 1:0:-1],
                    tb[:, SR - 2::-1], tb[:, SR - 1:SR],
                    op0=AL.max, op1=AL.min)
                # transpose back
                pout = psum.tile([128, SR], F32, tag="pout")
                for b in range(NB):
                    nc.tensor.transpose(pout[:, b * 128:(b + 1) * 128],
                                        tb[:, b * 128:(b + 1) * 128],
                                        ident[:])
                for b in range(NB):
                    nc.scalar.copy(LR[b][:, t * 128:(t + 1) * 128],
                                   pout[:, b * 128:(b + 1) * 128])

        with tc.For_i(0, NIT, 1):
            iteration()
        # snapshot, one more iteration, compare -> convergence flag
        for b in range(NB):
            nc.sync.dma_start(prev_r[b], LR[b][:])
        iteration()
        for b in range(NB):
            nc.sync.dma_start(t1[:], prev_r[b])
            nc.vector.tensor_tensor(t1[:], t1[:], LR[b][:], op=AL.not_equal)
            nc.vector.tensor_reduce(fr[:], t1[:], axis=AX.X, op=AL.max)
            nc.vector.tensor_tensor(flag[:], flag[:], fr[:], op=AL.max)

        # ---- epilogue: labels to u16 tile-local codes (bg -> 0), flag out --
        # code = dy*TX + dx + 1 where (dy, dx) = label position within its
        # 64x128 tile (tile-local solve => label lies inside the tile).
        HW2 = W // 2
        # LOCBASE = (p & 63)*W + (x & 127) into io[:, HW2:]
        nc.gpsimd.iota(io[:, HW2:], pattern=[[0, HW2]], base=0,
                       channel_multiplier=W)
        nc.vector.tensor_scalar(io[:, HW2:], io[:, HW2:], float((TY - 1) * W),
                                0.0, op0=AL.bitwise_and, op1=AL.bitwise_or)
        nc.gpsimd.iota(io[:, :HW2], pattern=[[1, HW2]], base=0,
                       channel_multiplier=0)
        nc.vector.tensor_scalar(io[:, :HW2], io[:, :HW2], float(TX - 1), 0.0,
                                op0=AL.bitwise_and, op1=AL.bitwise_or)
        nc.vector.tensor_tensor(io[:, HW2:], io[:, HW2:], io[:, :HW2],
                                op=AL.add)
        for b in range(NB):
            for h in range(2):
                sl = slice(h * HW2, (h + 1) * HW2)
                ta = (pool.tile([128, HW2], I32, name="ta")
                      if (b, h) == (0, 0) else ta)
                o16 = tmp.tile([128, HW2], U16, tag="o16")
                # label0 = label - 1 (i32)
                nc.vector.tensor_scalar(t1[:, :HW2], LR[b][:, sl], 1.0, 0.0,
                                        op0=AL.subtract, op1=AL.add)
                nc.vector.tensor_copy(io[:, :HW2], t1[:, :HW2])
                # tilebase = idx - LOCBASE ; local = label0 - tilebase
                nc.gpsimd.iota(ta[:], pattern=[[1, HW2]],
                               base=b * 128 * W + h * HW2,
                               channel_multiplier=W)
                nc.vector.tensor_tensor(ta[:], ta[:],
                                        offt[:].broadcast_to([128, HW2]),
                                        op=AL.add)
                nc.vector.tensor_tensor(ta[:], ta[:], io[:, HW2:],
                                        op=AL.subtract)
                nc.vector.tensor_tensor(io[:, :HW2], io[:, :HW2], ta[:],
                                        op=AL.subtract)
                # fg mask; code = (local>>12)*TX + (local&(TX-1)) + 1
                nc.vector.tensor_scalar(t1[:, :HW2], LR[b][:, sl], BIG, 0.0,
                                        op0=AL.is_lt, op1=AL.add)
                nc.vector.tensor_scalar(ta[:], io[:, :HW2],
                                        float(int(np.log2(W))), 0.0,
                                        op0=AL.logical_shift_right,
                                        op1=AL.bitwise_or)
                nc.vector.tensor_scalar(ta[:], ta[:], float(TX), 1.0,
                                        op0=AL.mult, op1=AL.add)
                nc.vector.tensor_scalar(io[:, :HW2], io[:, :HW2],
                                        float(TX - 1), 0.0,
                                        op0=AL.bitwise_and,
                                        op1=AL.bitwise_or)
                nc.vector.tensor_tensor(ta[:], ta[:], io[:, :HW2], op=AL.add)
                nc.vector.tensor_tensor(o16[:], ta[:], t1[:, :HW2],
                                        op=AL.mult)
                nc.sync.dma_start(lab_r[b][:, sl], o16[:])
        nc.sync.dma_start(outs["flag"], flag[:])


def build_program():
    nc = bacc.Bacc("TRN2", target_bir_lowering=False, debug=False,
                   num_devices=NCORES)
    ins = {
        "fgp": nc.dram_tensor("fgp", [SR, W // 8], U8,
                              kind="ExternalInput").ap(),
        "off": nc.dram_tensor("off", [128, 1], F32,
                              kind="ExternalInput").ap(),
    }
    outs = {
        "lab": nc.dram_tensor("lab", [SR, W], U16,
                              kind="ExternalOutput").ap(),
        "flag": nc.dram_tensor("flag", [128, 1], F32,
                               kind="ExternalOutput").ap(),
    }
    prev = nc.dram_tensor("prev", [SR, W], F32, kind="Internal").ap()
    with tile.TileContext(nc) as tc:
        kernel_body(tc, outs, ins, prev)
    nc.compile()
    return nc


# ---------------------------------------------------------------------------
# host side
# ---------------------------------------------------------------------------

def _host_finish(lab, fg):
    """Finish tile-local solves in numpy if the device flag fired (monotone
    continuation of the same operator; exact for any input)."""
    B = ~fg
    L = np.where(fg, lab.astype(np.int64), BIGI)
    gR = np.full((H, W), BIGI)
    m = fg[:, 1:] & fg[:, :-1]
    gR[:, 1:] = np.where(m, 0, BIGI)
    gR[:, 0::TX] = BIGI
    gC = np.full((H, W), BIGI)
    m = fg[1:, :] & fg[:-1, :]
    gC[1:, :] = np.where(m, 0, BIGI)
    for r in range(0, H, SR):   # strip borders are tile borders too
        gC[r, :] = BIGI
    gC[0::TY, :] = BIGI

    def scan(Lm, G, axis, reverse):
        if axis == 0:
            Lm = Lm.T; G = G.T
        h, w = Lm.shape
        if reverse:
            Lf = Lm[:, ::-1]
            Gf = np.empty_like(G)
            Gf[:, :w - 1] = G[:, 1:][:, ::-1]
            Gf[:, w - 1] = BIGI
        else:
            Lf, Gf = Lm, G
        seg = np.cumsum(Gf >= BIGI, axis=1)
        sp = (w + 2) - seg
        K = np.int64(1 << 32)
        C = np.minimum.accumulate(Lf + sp * K, axis=1)
        res = np.minimum(C - sp * K, Lf)
        if reverse:
            res = res[:, ::-1]
        return res.T if axis == 0 else res

    for _ in range(2000):
        prev = L
        L = scan(L, gR, 1, False)
        L = scan(L, gR, 1, True)
        Lh = L.reshape(H, W // TX, TX)
        a = np.full_like(Lh, BIGI); a[:, :, 1:] = Lh[:, :, :-1]
        b2 = np.full_like(Lh, BIGI); b2[:, :, :-1] = Lh[:, :, 1:]
        Hm = np.minimum(Lh, np.minimum(a, b2)).reshape(H, W)
        Hv = Hm.reshape(H // TY, TY, W)
        a = np.full_like(Hv, BIGI); a[:, 1:, :] = Hv[:, :-1, :]
        b2 = np.full_like(Hv, BIGI); b2[:, :-1, :] = Hv[:, 1:, :]
        L = np.where(B, BIGI, np.minimum(Hv, np.minimum(a, b2)).reshape(H, W))
        L = scan(L, gC, 0, False)
        L = scan(L, gC, 0, True)
        if np.array_equal(L, prev):
            break
    return np.where(B, 0, L).astype(np.int32)


def _merge(lab, fg):
    """Union tile-seam equivalences (pure numpy), remap to component minima."""
    pairs = []

    def add(la, lb, fa, fb):
        m = fa & fb
        if m.any():
            pairs.append(np.stack([la[m], lb[m]], 1))
        m = fa[1:] & fb[:-1]
        if m.any():
            pairs.append(np.stack([la[1:][m], lb[:-1][m]], 1))
        m = fa[:-1] & fb[1:]
        if m.any():
            pairs.append(np.stack([la[:-1][m], lb[1:][m]], 1))

    for r in range(TY - 1, H - 1, TY):
        add(lab[r], lab[r + 1], fg[r], fg[r + 1])
    for c in range(TX - 1, W - 1, TX):
        add(lab[:, c], lab[:, c + 1], fg[:, c], fg[:, c + 1])
    if not pairs:
        return lab
    P = np.concatenate(pairs)
    uniq, inv = np.unique(P, return_inverse=True)
    inv = inv.reshape(-1, 2)
    e0, e1 = inv[:, 0], inv[:, 1]
    par = np.arange(len(uniq), dtype=np.int64)
    while True:
        a = par[e0]; b = par[e1]
        if (a == b).all():
            break
        m = np.minimum(a, b)
        np.minimum.at(par, e0, m)
        np.minimum.at(par, e1, m)
        par = par[par[par]]
    LUT = np.arange(H * W + 1, dtype=np.int32)
    LUT[uniq] = uniq[par].astype(np.int32)
    return LUT[lab]


_CACHED = {}


def _build_runner(nc):
    """Compiled executor for the Bass program (modeled on
    bass2jax.run_bass_via_pjrt): same NEFF custom call via PJRT/shard_map,
    but the jitted callable is cached across launches and the output buffers
    are zero-filled on device instead of uploaded from host."""
    import jax
    from concourse import bass2jax
    from jax.experimental.shard_map import shard_map
    from jax.sharding import Mesh, PartitionSpec

    bass2jax.install_neuronx_cc_hook()
    partition_name = (nc.partition_id_tensor.name
                      if nc.partition_id_tensor else None)
    in_names, out_names, out_avals = [], [], []
    for alloc in nc.m.functions[0].allocations:
        if not isinstance(alloc, mybir.MemoryLocationSet):
            continue
        name = alloc.memorylocations[0].name
        if alloc.kind == "ExternalInput":
            if name != partition_name:
                in_names.append(name)
        elif alloc.kind == "ExternalOutput":
            out_names.append(name)
            out_avals.append(jax.core.ShapedArray(
                tuple(alloc.tensor_shape), mybir.dt.np(alloc.dtype)))
    n_params = len(in_names)
    bind_names = tuple(in_names + out_names +
                       ([partition_name] if partition_name else []))

    def _body(*args):
        operands = list(args)
        if partition_name is not None:
            operands.append(bass2jax.partition_id_tensor())
        outs = bass2jax._bass_exec_p.bind(
            *operands, out_avals=tuple(out_avals), in_names=bind_names,
            out_names=tuple(out_names), lowering_input_output_aliases=(),
            sim_require_finite=True, sim_require_nnan=True, nc=nc)
        return tuple(outs)

    n_outs = len(out_names)
    devices = jax.devices()[:NCORES]
    mesh = Mesh(np.asarray(devices), ("core",))
    sharded = jax.jit(
        shard_map(_body, mesh=mesh,
                  in_specs=(PartitionSpec("core"),) * (n_params + n_outs),
                  out_specs=(PartitionSpec("core"),) * n_outs,
                  check_rep=False),
        donate_argnums=tuple(range(n_params, n_params + n_outs)),
        keep_unused=True)
    from jax.sharding import NamedSharding
    shardings = tuple(NamedSharding(mesh, PartitionSpec("core"))
                      for _ in range(n_outs))
    zmaker = jax.jit(
        lambda: tuple(jax.numpy.zeros((NCORES * av.shape[0],) + av.shape[1:],
                                      av.dtype) for av in out_avals),
        out_shardings=shardings)

    import time as _time

    def run(per_core_maps):
        t0 = _time.time()
        concat_in = [
            np.concatenate([m[name] for m in per_core_maps], axis=0)
            for name in in_names]
        t1 = _time.time()
        out_arrs = sharded(*concat_in, *zmaker())
        jax.block_until_ready(out_arrs)
        t2 = _time.time()
        for a in out_arrs:
            a.copy_to_host_async()
        res = {}
        for i, name in enumerate(out_names):
            res[name] = np.asarray(out_arrs[i]).reshape(
                NCORES, *out_avals[i].shape)
        t3 = _time.time()
        run.phases = (t1 - t0, t2 - t1, t3 - t2)
        return res

    return run


def kernel(prob):
    prob2 = np.squeeze(np.asarray(prob))
    fg = prob2 > 0.5

    if 'run' not in _CACHED:
        _CACHED['nc'] = build_program()
        _CACHED['run'] = _build_runner(_CACHED['nc'])
        yy = (np.arange(H, dtype=np.int32) // TY) * TY
        xx = (np.arange(W, dtype=np.int32) // TX) * TX
        _CACHED['base_grid'] = (yy[:, None] * W + xx[None, :] + 1)
        c = np.arange(1 << 14, dtype=np.int32)
        off = ((c - 1) >> 7) * W + ((c - 1) & (TX - 1))
        off[0] = 0
        _CACHED['off_lut'] = off.astype(np.int32)

    fgp = np.packbits(fg, axis=1, bitorder='little')
    in_maps = []
    for c in range(NCORES):
        in_maps.append({
            "fgp": fgp[c * SR:(c + 1) * SR],
            "off": np.full((128, 1), float(c * SR * W), np.float32),
        })
    res = _CACHED['run'](in_maps)
    kernel._launches = 1

    code = res["lab"].reshape(H, W)
    lab = ((_CACHED['base_grid'] + _CACHED['off_lut'][code])
           * (code != 0)).astype(np.int32)
    converged = float(res["flag"].max()) == 0.0
    if not converged:
        lab = _host_finish(lab, fg)
    return _merge(lab, fg)
